# revision 29
# baseline (speedup 1.0000x reference)
"""Localized embedding layer (separable Gaussian stencil) on 8 trn2 cores.

Math: out[i,j,:] = sum_{|di|,|dj|<=2} w(di)w(dj) H[i+di,j+dj,:] / (r(i)r(j))
with w(d) = exp(-c*d^2), c = TILE^2/(2 sigma^2).  w(2) = 4.4e-5 contributes
~1e-4 relative -> drop the +-2 taps (3x3 stencil), well inside the 2e-2
tolerance.  All wire traffic is bf16 (cast host-side, untimed); compute is
bf16 with f32 PSUM accumulation.

Per core (32 output grid rows + 1-row halo, zero padded):
  - x resident in one SBUF buffer [128p=j-half, 34r, 2h, 512d] bf16,
    loaded as 17 two-row DMAs (~508KB each).
  - i-conv: 2 DVE ops per 8-row block over [128, 8*1024] bf16 (2x mode).
  - j-conv: TensorE banded 128x128 bf16 matmul per (row, half); column
    normalization 1/(r(j)*W3) folded into the band matrix.
  - PSUM drain: one ScalarE copy per 2-row psum tile (f32->bf16); global
    edge rows get their W3/r(i) scale via per-core consts.
  - stores skip columns 127/128 (their stencil crosses the half boundary);
    a strip pass recomputes them: i-conv on PE (34->32 banded matmul),
    j-conv as DVE free-dim shifts, one [32,2,512] store.
"""

import sys
import numpy as np

if "/opt/trn_rl_repo" not in sys.path:
    sys.path.insert(0, "/opt/trn_rl_repo")

G = 256          # grid side
D = 512          # feature dim
P = 1            # halo rows (3-tap stencil)
NC = 8           # cores
RPC = G // NC    # rows per core = 32
NR = RPC + 2 * P  # x rows per core = 34
TILE = 448.0
SIGMA = 200.0
BLK = 8          # i-conv block rows

_cache = {}


def _w1():
    c = TILE * TILE / (2.0 * SIGMA * SIGMA)
    return float(np.exp(-c))


def _r3():
    """r(j) = sum of valid 3-tap weights at position j."""
    w1 = _w1()
    r = np.full(G, 1.0 + 2.0 * w1)
    r[0] = r[-1] = 1.0 + w1
    return r


def _host_consts():
    import ml_dtypes

    bf16 = ml_dtypes.bfloat16
    w1 = _w1()
    W3 = 1.0 + 2.0 * w1
    r3 = _r3()
    taps = np.array([w1, 1.0, w1])
    # banded lhsT per half: wmat[k=jin_loc, hm, m=jout_loc]
    #   = w(m-k) / (r3(jout) * W3)
    wmat = np.zeros((128, 2, 128))
    for hm in range(2):
        for m in range(128):
            jout = m + 128 * hm
            for dd in (-1, 0, 1):
                k = m + dd
                if 0 <= k < 128:
                    wmat[k, hm, m] = taps[dd + 1] / (r3[jout] * W3)
    # strip i-conv lhsT [34, 32]: ts[i] = w1*x[i] + x[i+1] + w1*x[i+2]
    wstrip = np.zeros((NR, RPC))
    for i in range(RPC):
        for k in range(3):
            wstrip[i + k, i] = taps[k]
    # per-core row scales
    se_l, sf_l = [], []
    for c in range(NC):
        rows = r3[RPC * c: RPC * (c + 1)]
        se = np.ones((128, 2), dtype=np.float32)
        se[:, 0] = W3 / rows[0]
        se[:, 1] = W3 / rows[-1]
        sf = np.zeros((128, 1), dtype=np.float32)
        sf[:RPC, 0] = (1.0 / (W3 * rows)).astype(np.float32)
        se_l.append(se)
        sf_l.append(sf)
    return wmat.astype(bf16), (w1 * wmat).astype(bf16), wstrip.astype(bf16), se_l, sf_l


def _build_nc():
    import concourse.bass as bass  # noqa: F401
    import concourse.mybir as mybir
    import concourse.tile as tile
    from concourse import bacc

    f32 = mybir.dt.float32
    bf = mybir.dt.bfloat16
    add = mybir.AluOpType.add
    mult = mybir.AluOpType.mult

    w1 = _w1()

    nc = bacc.Bacc(None, target_bir_lowering=False, debug=False)
    x_dram = nc.declare_dram_parameter("x", [NR, G, D], bf, isOutput=False)
    wm_dram = nc.declare_dram_parameter("wmat", [128, 2, 128], bf, isOutput=False)
    wm2_dram = nc.declare_dram_parameter("wmat2", [128, 2, 128], bf, isOutput=False)
    wf_dram = nc.declare_dram_parameter("wstrip", [NR, RPC], bf, isOutput=False)
    se_dram = nc.declare_dram_parameter("sedge", [128, 2], f32, isOutput=False)
    sf_dram = nc.declare_dram_parameter("sfix", [128, 1], f32, isOutput=False)
    # i-major output.  HW-probed rule: the SBUF side of a DMA must cover
    # EXACTLY 128 partitions, else most descriptors land on one SDMA engine
    # (~24 GB/s); with 128 partitions they spread across all 16 engines.
    # So each store writes a full 128-column half (garbage at jout 127/128);
    # the strip pass writes those two columns to yfix, patched on the host.
    y_dram = nc.declare_dram_parameter("y", [2, 128, RPC, D], bf, isOutput=True)
    yf_dram = nc.declare_dram_parameter("yfix", [RPC, 2, D], bf, isOutput=True)

    NPAIR = NR // 2  # 17

    with tile.TileContext(nc) as tc:
        with (
            tc.tile_pool(name="const", bufs=1) as cpool,
            tc.tile_pool(name="x", bufs=1) as xpool,
            tc.tile_pool(name="t", bufs=2) as tpool,
            tc.tile_pool(name="out", bufs=6) as opool,
            tc.tile_pool(name="fix", bufs=1) as fpool,
            tc.tile_pool(name="psum", bufs=2, space="PSUM") as ppool,
        ):
            wt = cpool.tile([128, 2, 128], bf)
            nc.sync.dma_start(wt[:], wm_dram[:])
            wt2 = cpool.tile([128, 2, 128], bf)
            nc.sync.dma_start(wt2[:], wm2_dram[:])
            wft = cpool.tile([NR, RPC], bf)
            nc.sync.dma_start(wft[:], wf_dram[:])
            se = cpool.tile([128, 2], f32)
            nc.sync.dma_start(se[:], se_dram[:])
            sft = cpool.tile([128, 1], f32)
            nc.sync.dma_start(sft[:], sf_dram[:])

            xbig = xpool.tile([128, NR, 2, D], bf)
            xs = fpool.tile([NR, 4, D], bf, tag="xs")

            loaded = set()

            def load_pair(m):
                if m in loaded or m >= NPAIR:
                    return
                loaded.add(m)
                r0 = 2 * m
                nc.sync.dma_start(
                    xbig[:, r0:r0 + 2, :, :],
                    x_dram[r0:r0 + 2].rearrange("r (h p) d -> p r h d", p=128),
                )

            # graduated block sizes: small first blocks start the PE/ACT/store
            # pipeline early; big later blocks amortize DVE op overhead
            BLOCKS = [(0, 2), (2, 2), (4, 4), (8, 8), (16, 8), (24, 4),
                      (28, 2), (30, 2)]

            def pairs_for(bi):
                r0, n = BLOCKS[bi]
                return range(r0 // 2, (r0 + n + 1) // 2 + 1)

            for m in pairs_for(0):
                load_pair(m)
            # strip columns 126..129 for all 34 rows (boundary fix input)
            nc.sync.dma_start(xs[:], x_dram[:, 126:130, :])

            def strip_pass():
                # strip pass: recompute jout 126..129 -> only 127,128 needed
                # (borrows a "ps" slot; runs mid-kernel so its serial chain
                # overlaps the main pipeline instead of tailing it)
                psf = ppool.tile([128, 2, 2, D], f32, tag="ps", name="psf")
                for cix in range(4):
                    nc.tensor.matmul(
                        psf[0:RPC, cix // 2, cix % 2, :], wft[:], xs[:, cix, :],
                        start=True, stop=True,
                    )
                ts = fpool.tile([RPC, 4, D], bf, tag="ts", name="ts")
                nc.scalar.copy(
                    ts[:], psf[0:RPC, :, :, :].rearrange("p a b d -> p (a b) d")
                )
                gt = fpool.tile([RPC, 2, D], bf, tag="g", name="gt")
                nc.vector.tensor_tensor(gt[:], ts[:, 0:2, :], ts[:, 2:4, :], add)
                f = fpool.tile([RPC, 2, D], bf, tag="f", name="f")
                nc.vector.scalar_tensor_tensor(
                    f[:], gt[:], w1, ts[:, 1:3, :], mult, add
                )
                fs = fpool.tile([RPC, 2, D], bf, tag="fs", name="fs")
                nc.vector.tensor_scalar(fs[:], f[:], sft[0:RPC, 0:1], None, mult)
                nc.sync.dma_start(yf_dram[:], fs[:])

            pair_idx = 0
            for b, (r, n) in enumerate(BLOCKS):
                if b + 1 < len(BLOCKS):
                    for m in pairs_for(b + 1):
                        load_pair(m)
                # i-conv: t[i] = w1*(x[r+i] + x[r+i+2]) + x[r+i+1]
                # TT (2x mode) + TS (4x mode) + TT: avoids 1x-mode STT.
                # Late blocks (route B): DVE does only the TT; the w1-fold
                # rides the PE as a second accumulating matmul with w1*B.
                route_b = r >= 16
                u = tpool.tile([128, n, 2, D], bf, tag="u")
                nc.vector.tensor_tensor(
                    u[:], xbig[:, r:r + n, :, :], xbig[:, r + 2:r + n + 2, :, :], add
                )
                if not route_b:
                    nc.vector.tensor_scalar(u[:], u[:], w1, None, mult)
                    t = tpool.tile([128, n, 2, D], bf, tag="t")
                    nc.vector.tensor_tensor(
                        t[:], u[:], xbig[:, r + 1:r + n + 1, :, :], add
                    )
                for r2 in range(n // 2):         # 2-row psum tiles
                    i0 = r + 2 * r2              # output row pair (i0, i0+1)
                    ps = ppool.tile([128, 2, 2, D], f32, tag="ps")
                    for hm in range(2):
                        for dr in range(2):
                            if route_b:
                                nc.tensor.matmul(
                                    ps[:, hm, dr, :],
                                    wt[:, hm, :],
                                    xbig[:, r + 2 * r2 + dr + 1, hm, :],
                                    start=True,
                                    stop=False,
                                )
                                nc.tensor.matmul(
                                    ps[:, hm, dr, :],
                                    wt2[:, hm, :],
                                    u[:, 2 * r2 + dr, hm, :],
                                    start=False,
                                    stop=True,
                                )
                            else:
                                nc.tensor.matmul(
                                    ps[:, hm, dr, :],
                                    wt[:, hm, :],
                                    t[:, 2 * r2 + dr, hm, :],
                                    start=True,
                                    stop=True,
                                )
                    ob = opool.tile([128, 2, 2, D], bf, tag="ob")
                    if i0 == 0:
                        # edge row is dr=0: (hm, d) slices, scale se0
                        nc.scalar.mul(ob[:, :, 0, :], ps[:, :, 0, :], se[:, 0:1])
                        nc.scalar.copy(ob[:, :, 1, :], ps[:, :, 1, :])
                    elif i0 == RPC - 2:
                        # last drain on DVE (idle by then); edge row via TS
                        nc.vector.tensor_copy(ob[:], ps[:])
                        nc.vector.tensor_scalar(
                            ob[:, :, 1, :], ob[:, :, 1, :], se[:, 1:2], None, mult
                        )
                    elif i0 >= 28:
                        nc.vector.tensor_copy(ob[:], ps[:])
                    else:
                        nc.scalar.copy(ob[:], ps[:])
                    # j-major half-split stores: 2-dim ([[16384,128],[1,1024]]),
                    # 128 partitions, 2KB descriptors.  Stores MUST be <=2-dim
                    # after balancing AND cover 128 partitions: 3-dim stores
                    # scatter on BOTH DGE paths; <128 partitions -> 1 engine.
                    eng = nc.gpsimd if pair_idx % 2 == 0 else nc.sync
                    pair_idx += 1
                    for hm in range(2):
                        eng.dma_start(
                            y_dram[hm, :, i0:i0 + 2, :], ob[:, hm, :, :]
                        )
                if b == 6:
                    strip_pass()

    nc.finalize()
    return nc


def _get_program():
    if "nc" not in _cache:
        _cache["nc"] = _build_nc()
        _cache["consts"] = _host_consts()
    return _cache["nc"], _cache["consts"]


def _make_in_maps(H):
    import ml_dtypes

    bf16 = ml_dtypes.bfloat16
    nc, (wmat, wmat2, wstrip, se_l, sf_l) = _get_program()
    H3 = H.reshape(G, G, D).astype(bf16)
    Hp = np.zeros((G + 2 * P, G, D), dtype=bf16)
    Hp[P:P + G] = H3
    in_maps = []
    for c in range(NC):
        shard = np.ascontiguousarray(Hp[RPC * c: RPC * c + NR])
        in_maps.append(
            {"x": shard, "wmat": wmat, "wmat2": wmat2, "wstrip": wstrip,
             "sedge": se_l[c], "sfix": sf_l[c]}
        )
    return nc, in_maps


def _assemble(res):
    parts = []
    for c in range(NC):
        y2 = np.asarray(res[c]["y"]).reshape(2, 128, RPC, D)
        y = np.ascontiguousarray(np.transpose(y2, (2, 0, 1, 3))).reshape(RPC, G, D)
        yfix = np.asarray(res[c]["yfix"]).reshape(RPC, 2, D)
        y[:, 127:129, :] = yfix
        parts.append(y.reshape(RPC * G, D))
    return np.concatenate(parts, axis=0).astype(np.float32)


def kernel(H, xy=None):
    from concourse.bass_utils import run_bass_kernel_spmd

    nc, in_maps = _make_in_maps(H)
    res = run_bass_kernel_spmd(nc, in_maps, list(range(NC))).results
    return _assemble(res)
